# Initial kernel scaffold
#
"""MoE dispatcher kernel for Trainium2 (8 NeuronCores, expert-parallel).

Contract: kernel(**inputs) takes FULL inputs and returns the FULL output.

Strategy (expert-parallel, matches the sharding hint):
  - host: softmax(gate_logits) -> top-2 -> combine weights per (token, expert)
  - host "all-to-all dispatch": for expert e, gather its routed tokens,
    pre-scale rows by the combine weight (w * (x @ W) == (w*x) @ W), pad to a
    common capacity C, transpose to [D, C] so the device streams tokens along
    the free dim.  One expert per core.
  - device (per core): Y^T[D,C] = W[e]^T @ X^T via PE array, tiled
    [128 x <=512] PSUM accumulation over K=D.
  - host "all-to-all combine": scatter-add each expert's Y rows back to the
    token axis (plain add; weights were folded into x).

Perf notes (traced on HW, ~48us from ~55us baseline):
  - W chunks and the j>=1 x tiles share the sync queue, in consumption
    order; putting the j>=1 x tiles on the otherwise-idle scalar queue let
    them steal HBM bandwidth from the last W chunks and stalled j=0.
  - bf16 outputs halve the output-DMA drain after the last matmul.
  - capacity C is the exact max expert load (no 128 rounding); the
    remainder over 512 splits evenly (tiles under ~257 cols are bound by
    the ~107ns per-matmul instruction floor, so [512,309,308] beats
    [512,512,105]).
  - warmup matmuls ramp the HAM clock gate (PE runs at 4/8 for ~6us from
    first PE activity); tail dummy matmuls keep it at 8/8 into the
    runtime's fixed ~5-7us teardown (sem-clear chain on the tensor queue).
  - the NEFF epilogue (all-engine sem clear) and the ~5us head (DMA queue
    spin-up + first W/X chunk) are fixed costs outside program control.

DRAM layouts are host-permuted so every DMA is fully contiguous per
partition:
  w   [P, KT*D]            w[p, k*D + d] = W[e][k*128 + p, d]
  xt  [NT, P, KT*NSPLIT]   xt[j, p, k*nsz_j + n] = X^T[k*128 + p, n0_j + n]
  yt  [MT, P, C]           yt[m, p, n]   = Y^T[m*128 + p, n]
"""

import os

import numpy as np

N_CORES = 8
P = 128
NSPLIT = 512  # max moving-operand / PSUM-bank free dim (fp32)
NMIN = 256  # keep moving tiles >=256 wide (f32r runs 4 cyc/row below 256)
PASSES = 8  # k-dim chunks for W / first-n-tile pipelining

# matmul input dtype: "float32", "float32r", or "bfloat16"
MM_DT = os.environ.get("BASS_MOE_DT", "bfloat16")
# device output dtype (bf16 halves output DMA; rel-err stays ~3e-3)
OUT_DT = os.environ.get("BASS_MOE_OUT_DT", "bfloat16")
# 10 warmup matmuls: covers per-core data-arrival variance (~10.1-11.8us)
# so the straggler core's PE never idles mid-ramp — trades ~0.4us of mean
# for the MAX core, which is the graded metric
WARMUP_MM = int(os.environ.get("BASS_MOE_WARMUP", "10"))
# dummy matmuls after the last real one: keep the HAM clock gate at 8/8
# through the output-DMA drain + runtime epilogue (whose tensor-engine
# semaphore-clear chain otherwise runs at 4/8 half clock)
TAIL_MM = int(os.environ.get("BASS_MOE_TAIL", "12"))
# output queue policy: "alt" (alternate sync/scalar) or "sync"
OUT_Q = os.environ.get("BASS_MOE_OUT_Q", "alt")
# n-tile policy: "even" (split remainder evenly) or "aligned" (512/384/256)
NT_MODE = os.environ.get("BASS_MOE_NT", "even")
# capacity: "exact" or "128" (round up to multiple of 128)
C_MODE = os.environ.get("BASS_MOE_C", "exact")
# 1: memset the warmup tile (adds ~0.8us before the first warmup matmul);
# 0: warmup reads uninitialized SBUF (garbage is harmless — the warmup
#    PSUM is never read) so the HAM clock ramp starts right after the
#    preamble barrier
WARM_INIT = int(os.environ.get("BASS_MOE_WARM_INIT", "1"))

_prog_cache: dict = {}


def _np_dt(name):
    if name == "bfloat16":
        import ml_dtypes

        return ml_dtypes.bfloat16
    return np.float32


def _n_tiles(C):
    """Split C into tiles of at most NSPLIT.

    Tiles narrower than ~257 cols are bound by the per-instruction floor
    (LDWEIGHTS + issue overhead ~107ns), so when the remainder exceeds
    NSPLIT split it evenly instead of NSPLIT + narrow leftover.
    """
    out = []
    rem = C
    n0 = 0
    while rem > 0:
        if NT_MODE == "aligned":
            if rem > NSPLIT + NMIN // 2:
                sz = NSPLIT
            elif rem > NSPLIT:
                sz = (rem // 2 + P - 1) // P * P
            else:
                sz = rem
        else:
            if rem > 2 * NSPLIT:
                sz = NSPLIT
            elif rem > NSPLIT:
                sz = (rem + 1) // 2
            else:
                sz = rem
        out.append((n0, sz))
        n0 += sz
        rem -= sz
    return out


def _build_program(D: int, C: int, mm_dt_name: str, out_dt_name: str):
    import concourse.bacc as bacc
    import concourse.mybir as mybir
    import concourse.tile as tile

    mm_dt = getattr(mybir.dt, mm_dt_name)
    out_dt = getattr(mybir.dt, out_dt_name)
    KT = D // P  # k tiles (contraction)
    MT = D // P  # m tiles (output features)
    KC = KT // PASSES  # k tiles per chunk
    n_tiles = _n_tiles(C)
    NT = len(n_tiles)

    nc = bacc.Bacc(None, target_bir_lowering=False)
    xt = nc.declare_dram_parameter("xt", [NT, P, KT * NSPLIT], mm_dt, isOutput=False)
    w = nc.declare_dram_parameter("w", [P, KT * D], mm_dt, isOutput=False)
    yt = nc.declare_dram_parameter("yt", [MT, P, C], out_dt, isOutput=True)

    with tile.TileContext(nc) as tc:
        with (
            tc.tile_pool(name="wpool", bufs=PASSES) as wpool,
            tc.tile_pool(name="xpool", bufs=PASSES) as xpool,
            tc.tile_pool(name="psum", bufs=8, space="PSUM") as psum_pool,
            tc.tile_pool(name="opool", bufs=4) as opool,
            tc.tile_pool(name="warm", bufs=1) as warmpool,
        ):
            if WARMUP_MM:
                # Keep the PE busy during the DMA lead-in so the HAM clock
                # gate is at 8/8 when the real matmuls start.
                wt = warmpool.tile([P, NSPLIT], mybir.dt.bfloat16, tag="warm_w")
                if WARM_INIT == 1:
                    nc.vector.memset(wt[:], 0.0)
                elif WARM_INIT == 2:
                    # minimal write: allocates the tile and covers the lhsT
                    # read; the rhs columns stay uninitialized (their product
                    # lands in a PSUM tile nothing reads)
                    nc.vector.memset(wt[:, :P], 0.0)
                for i in range(WARMUP_MM):
                    wp = psum_pool.tile([P, NSPLIT], mybir.dt.float32, tag="ps")
                    nc.tensor.matmul(
                        wp[:], lhsT=wt[:, :P], rhs=wt[:], start=True, stop=True
                    )

            # All input DMAs on the sync queue, in consumption order (the
            # queue drains FIFO).  W and the first n-tile of X are split into
            # PASSES interleaved k-chunks so the first matmul pass starts
            # after ~1/PASSES of the critical bytes have landed.
            n0_first, nsz_first = n_tiles[0]
            w_sb = []
            x0_sb = []
            for c in range(PASSES):
                tw = wpool.tile([P, KC, D], mm_dt, tag="w_sb")
                nc.sync.dma_start(
                    tw[:].rearrange("p k d -> p (k d)"),
                    w[:, c * KC * D : (c + 1) * KC * D],
                )
                w_sb.append(tw)
                tx = xpool.tile([P, KC, nsz_first], mm_dt, tag="x0_sb")
                nc.scalar.dma_start(
                    tx[:].rearrange("p k n -> p (k n)"),
                    xt[0, :, c * KC * nsz_first : (c + 1) * KC * nsz_first],
                )
                x0_sb.append(tx)
            # j>=1 x tiles go on the SYNC queue *after* all W chunks: they
            # are not consumed until after the j=0 tile finishes, and putting
            # them on the (otherwise idle) scalar queue lets them steal HBM
            # bandwidth from the late W chunks, stalling j=0's last k-passes.
            x_sb = [None]
            for j, (n0, nsz) in enumerate(n_tiles):
                if j == 0:
                    continue
                t = xpool.tile([P, KT, nsz], mm_dt, tag="x_sb")
                nc.sync.dma_start(
                    t[:].rearrange("p k n -> p (k n)"),
                    xt[j, :, : KT * nsz],
                )
                x_sb.append(t)

            def mm(ps, mi, j, k, nsz, start, stop):
                if j == 0:
                    rhs = x0_sb[k // KC][:, k % KC, :nsz]
                else:
                    rhs = x_sb[j][:, k, :nsz]
                nc.tensor.matmul(
                    ps[:, :nsz],
                    lhsT=w_sb[k // KC][:, k % KC, mi * P : (mi + 1) * P],
                    rhs=rhs,
                    start=start,
                    stop=stop,
                )

            evict_n = [0]

            def evict(ps, mi, n0, nsz):
                ot = opool.tile([P, NSPLIT], out_dt, tag="ot")
                nc.vector.tensor_copy(ot[:, :nsz], ps[:, :nsz])
                # Alternate output queues: parallel trigger streams and a
                # two-queue final drain.
                if OUT_Q == "alt":
                    q = nc.sync if evict_n[0] % 2 == 0 else nc.scalar
                else:
                    q = nc.sync
                evict_n[0] += 1
                q.dma_start(yt[mi, :, n0 : n0 + nsz], ot[:, :nsz])

            # First n-tile: PASSES k-passes across all m, tracking the
            # arriving W/X0 chunks; evict on the last pass.
            ps0 = []
            for c in range(PASSES):
                for mi in range(MT):
                    if c == 0:
                        ps = psum_pool.tile([P, NSPLIT], mybir.dt.float32, tag="ps")
                        ps0.append(ps)
                    ps = ps0[mi]
                    for k in range(c * KC, (c + 1) * KC):
                        mm(
                            ps,
                            mi,
                            0,
                            k,
                            nsz_first,
                            start=(k == 0),
                            stop=(k == KT - 1),
                        )
                    if c == PASSES - 1:
                        evict(ps, mi, n0_first, nsz_first)
            # Remaining n-tiles: fused k loop.
            for j, (n0, nsz) in enumerate(n_tiles):
                if j == 0:
                    continue
                for mi in range(MT):
                    ps = psum_pool.tile([P, NSPLIT], mybir.dt.float32, tag="ps")
                    for k in range(KT):
                        mm(ps, mi, j, k, nsz, start=(k == 0), stop=(k == KT - 1))
                    evict(ps, mi, n0, nsz)
            if WARMUP_MM and TAIL_MM:
                # Keep the PE active through the output-DMA drain so the HAM
                # clock gate stays at 8/8 for the runtime's teardown sequence.
                tp = psum_pool.tile([P, NSPLIT], mybir.dt.float32, tag="ps")
                for i in range(TAIL_MM):
                    nc.tensor.matmul(
                        tp[:], lhsT=wt[:, :P], rhs=wt[:], start=True, stop=True
                    )
    nc.compile()
    return nc


def kernel(hidden: np.ndarray, gate_logits: np.ndarray, W: np.ndarray) -> np.ndarray:
    from concourse.bass_utils import run_bass_kernel_spmd

    hidden = np.asarray(hidden)
    gate_logits = np.asarray(gate_logits)
    W = np.asarray(W)
    B, S, D = hidden.shape
    T, E = gate_logits.shape
    assert E == N_CORES
    x = np.ascontiguousarray(hidden.reshape(T, D).astype(np.float32))

    # --- routing on host (fp32, matches reference softmax/top-2) ---
    g = gate_logits.astype(np.float32)
    m = g.max(axis=-1, keepdims=True)
    p = np.exp(g - m)
    p /= p.sum(axis=-1, keepdims=True)
    top2 = np.argpartition(-p, 1, axis=-1)[:, :2]

    routed = [np.nonzero((top2 == e).any(axis=1))[0] for e in range(E)]
    counts = np.array([len(r) for r in routed])
    if C_MODE == "exact":
        C = max(NMIN, int(counts.max()))
    else:
        C = max(NMIN, int(-(-counts.max() // P)) * P)

    mm_np = _np_dt(MM_DT)
    KT = D // P
    n_tiles = _n_tiles(C)
    NT = len(n_tiles)

    in_maps = []
    for e in range(E):
        idx = routed[e]
        scale = p[idx, e].astype(np.float32)
        xe = x[idx] * scale[:, None]  # [cnt, D]
        xt_full = np.zeros((D, C), dtype=mm_np)
        xt_full[:, : len(idx)] = xe.T.astype(mm_np)
        # [D, C] -> [KT, P, C] -> per-n-tile [NT, P, KT, nsz] contiguous
        xk = xt_full.reshape(KT, P, C)
        xt_dram = np.zeros((NT, P, KT * NSPLIT), dtype=mm_np)
        for j, (n0, nsz) in enumerate(n_tiles):
            blk = xk[:, :, n0 : n0 + nsz].transpose(1, 0, 2)  # [P, KT, nsz]
            xt_dram[j, :, : KT * nsz] = blk.reshape(P, KT * nsz)
        w_full = (
            W[e].astype(mm_np).reshape(KT, P, D).transpose(1, 0, 2).reshape(P, KT * D)
        )
        in_maps.append(
            {
                "xt": np.ascontiguousarray(xt_dram),
                "w": np.ascontiguousarray(w_full),
            }
        )

    key = (D, C, MM_DT, OUT_DT, WARMUP_MM, TAIL_MM, OUT_Q, NT_MODE, WARM_INIT)
    if key not in _prog_cache:
        _prog_cache[key] = _build_program(D, C, MM_DT, OUT_DT)
    nc = _prog_cache[key]

    res = run_bass_kernel_spmd(nc, in_maps, core_ids=list(range(N_CORES)))

    # --- combine on host ---
    out = np.zeros((T, D), dtype=np.float32)
    for e in range(E):
        idx = routed[e]
        ye_t = res.results[e]["yt"].reshape(D, C)  # Y^T
        out[idx] += ye_t[:, : len(idx)].T.astype(np.float32)
    return out.reshape(B, S, D)



# revision 1
# speedup vs baseline: 1.1427x; 1.1427x over previous
"""MoE dispatcher kernel for Trainium2 (8 NeuronCores, expert-parallel).

Contract: kernel(**inputs) takes FULL inputs and returns the FULL output.

Strategy (expert-parallel, matches the sharding hint):
  - host: softmax(gate_logits) -> top-2 -> combine weights per (token, expert)
  - host "all-to-all dispatch": for expert e, gather its routed tokens,
    pre-scale rows by the combine weight (w * (x @ W) == (w*x) @ W), pad to a
    common capacity C, transpose to [D, C] so the device streams tokens along
    the free dim.  One expert per core.
  - device (per core): Y^T[D,C] = W[e]^T @ X^T via PE array, tiled
    [128 x <=512] PSUM accumulation over K=D.
  - host "all-to-all combine": scatter-add each expert's Y rows back to the
    token axis (plain add; weights were folded into x).

Perf notes (traced on HW, ~48us from ~55us baseline):
  - W chunks and the j>=1 x tiles share the sync queue, in consumption
    order; putting the j>=1 x tiles on the otherwise-idle scalar queue let
    them steal HBM bandwidth from the last W chunks and stalled j=0.
  - bf16 outputs halve the output-DMA drain after the last matmul.
  - capacity C is the exact max expert load (no 128 rounding); the
    remainder over 512 splits evenly (tiles under ~257 cols are bound by
    the ~107ns per-matmul instruction floor, so [512,309,308] beats
    [512,512,105]).
  - warmup matmuls ramp the HAM clock gate (PE runs at 4/8 for ~6us from
    first PE activity); tail dummy matmuls keep it at 8/8 into the
    runtime's fixed ~5-7us teardown (sem-clear chain on the tensor queue).
  - the NEFF epilogue (all-engine sem clear) and the ~5us head (DMA queue
    spin-up + first W/X chunk) are fixed costs outside program control.

DRAM layouts are host-permuted so every DMA is fully contiguous per
partition:
  w   [P, KT*D]            w[p, k*D + d] = W[e][k*128 + p, d]
  xt  [NT, P, KT*NSPLIT]   xt[j, p, k*nsz_j + n] = X^T[k*128 + p, n0_j + n]
  yt  [MT, P, C]           yt[m, p, n]   = Y^T[m*128 + p, n]
"""

import os

import numpy as np

N_CORES = 8
P = 128
NSPLIT = 512  # max moving-operand / PSUM-bank free dim (fp32)
NMIN = 256  # keep moving tiles >=256 wide (f32r runs 4 cyc/row below 256)
PASSES = 8  # k-dim chunks for W / first-n-tile pipelining

# matmul input dtype: "float32", "float32r", or "bfloat16"
MM_DT = os.environ.get("BASS_MOE_DT", "bfloat16")
# device output dtype (bf16 halves output DMA; rel-err stays ~3e-3)
OUT_DT = os.environ.get("BASS_MOE_OUT_DT", "bfloat16")
# 10 warmup matmuls: covers per-core data-arrival variance (~10.1-11.8us)
# so the straggler core's PE never idles mid-ramp — trades ~0.4us of mean
# for the MAX core, which is the graded metric
WARMUP_MM = int(os.environ.get("BASS_MOE_WARMUP", "10"))
# dummy matmuls after the last real one: keep the HAM clock gate at 8/8
# through the output-DMA drain + runtime epilogue (whose tensor-engine
# semaphore-clear chain otherwise runs at 4/8 half clock)
TAIL_MM = int(os.environ.get("BASS_MOE_TAIL", "12"))
# output queue policy: "alt" (alternate sync/scalar) or "sync"
OUT_Q = os.environ.get("BASS_MOE_OUT_Q", "alt")
# n-tile policy: "even" (split remainder evenly) or "aligned" (512/384/256)
NT_MODE = os.environ.get("BASS_MOE_NT", "even")
# capacity: "exact" or "128" (round up to multiple of 128)
C_MODE = os.environ.get("BASS_MOE_C", "exact")
# 1: memset the warmup tile (adds ~0.8us before the first warmup matmul);
# 0: warmup reads uninitialized SBUF (garbage is harmless — the warmup
#    PSUM is never read) so the HAM clock ramp starts right after the
#    preamble barrier
WARM_INIT = int(os.environ.get("BASS_MOE_WARM_INIT", "1"))

_prog_cache: dict = {}


def _np_dt(name):
    if name == "bfloat16":
        import ml_dtypes

        return ml_dtypes.bfloat16
    return np.float32


def _n_tiles(C):
    """Split C into tiles of at most NSPLIT.

    Tiles narrower than ~257 cols are bound by the per-instruction floor
    (LDWEIGHTS + issue overhead ~107ns), so when the remainder exceeds
    NSPLIT split it evenly instead of NSPLIT + narrow leftover.
    """
    out = []
    rem = C
    n0 = 0
    while rem > 0:
        if NT_MODE == "aligned":
            if rem > NSPLIT + NMIN // 2:
                sz = NSPLIT
            elif rem > NSPLIT:
                sz = (rem // 2 + P - 1) // P * P
            else:
                sz = rem
        else:
            if rem > 2 * NSPLIT:
                sz = NSPLIT
            elif rem > NSPLIT:
                sz = (rem + 1) // 2
            else:
                sz = rem
        out.append((n0, sz))
        n0 += sz
        rem -= sz
    return out


def _build_program(D: int, C: int, mm_dt_name: str, out_dt_name: str):
    import concourse.bacc as bacc
    import concourse.mybir as mybir
    import concourse.tile as tile

    mm_dt = getattr(mybir.dt, mm_dt_name)
    out_dt = getattr(mybir.dt, out_dt_name)
    KT = D // P  # k tiles (contraction)
    MT = D // P  # m tiles (output features)
    KC = KT // PASSES  # k tiles per chunk
    n_tiles = _n_tiles(C)
    NT = len(n_tiles)

    nc = bacc.Bacc(None, target_bir_lowering=False)
    xt = nc.declare_dram_parameter("xt", [NT, P, KT * NSPLIT], mm_dt, isOutput=False)
    w = nc.declare_dram_parameter("w", [P, KT * D], mm_dt, isOutput=False)
    yt = nc.declare_dram_parameter("yt", [MT, P, C], out_dt, isOutput=True)

    with tile.TileContext(nc) as tc:
        with (
            tc.tile_pool(name="wpool", bufs=PASSES) as wpool,
            tc.tile_pool(name="xpool", bufs=PASSES) as xpool,
            tc.tile_pool(name="psum", bufs=8, space="PSUM") as psum_pool,
            tc.tile_pool(name="opool", bufs=4) as opool,
            tc.tile_pool(name="warm", bufs=1) as warmpool,
        ):
            if WARMUP_MM:
                # Keep the PE busy during the DMA lead-in so the HAM clock
                # gate is at 8/8 when the real matmuls start.
                wt = warmpool.tile([P, NSPLIT], mybir.dt.bfloat16, tag="warm_w")
                if WARM_INIT == 1:
                    nc.vector.memset(wt[:], 0.0)
                elif WARM_INIT == 2:
                    # minimal write: allocates the tile and covers the lhsT
                    # read; the rhs columns stay uninitialized (their product
                    # lands in a PSUM tile nothing reads)
                    nc.vector.memset(wt[:, :P], 0.0)
                for i in range(WARMUP_MM):
                    wp = psum_pool.tile([P, NSPLIT], mybir.dt.float32, tag="ps")
                    nc.tensor.matmul(
                        wp[:], lhsT=wt[:, :P], rhs=wt[:], start=True, stop=True
                    )

            # All input DMAs on the sync queue, in consumption order (the
            # queue drains FIFO).  W and the first n-tile of X are split into
            # PASSES interleaved k-chunks so the first matmul pass starts
            # after ~1/PASSES of the critical bytes have landed.
            n0_first, nsz_first = n_tiles[0]
            w_sb = []
            x0_sb = []
            for c in range(PASSES):
                tw = wpool.tile([P, KC, D], mm_dt, tag="w_sb")
                nc.sync.dma_start(
                    tw[:].rearrange("p k d -> p (k d)"),
                    w[:, c * KC * D : (c + 1) * KC * D],
                )
                w_sb.append(tw)
                tx = xpool.tile([P, KC, nsz_first], mm_dt, tag="x0_sb")
                nc.scalar.dma_start(
                    tx[:].rearrange("p k n -> p (k n)"),
                    xt[0, :, c * KC * nsz_first : (c + 1) * KC * nsz_first],
                )
                x0_sb.append(tx)
            # j>=1 x tiles go on the SYNC queue *after* all W chunks: they
            # are not consumed until after the j=0 tile finishes, and putting
            # them on the (otherwise idle) scalar queue lets them steal HBM
            # bandwidth from the late W chunks, stalling j=0's last k-passes.
            x_sb = [None]
            for j, (n0, nsz) in enumerate(n_tiles):
                if j == 0:
                    continue
                t = xpool.tile([P, KT, nsz], mm_dt, tag="x_sb")
                nc.sync.dma_start(
                    t[:].rearrange("p k n -> p (k n)"),
                    xt[j, :, : KT * nsz],
                )
                x_sb.append(t)

            def mm(ps, mi, j, k, nsz, start, stop):
                if j == 0:
                    rhs = x0_sb[k // KC][:, k % KC, :nsz]
                else:
                    rhs = x_sb[j][:, k, :nsz]
                nc.tensor.matmul(
                    ps[:, :nsz],
                    lhsT=w_sb[k // KC][:, k % KC, mi * P : (mi + 1) * P],
                    rhs=rhs,
                    start=start,
                    stop=stop,
                )

            evict_n = [0]

            def evict(ps, mi, n0, nsz):
                ot = opool.tile([P, NSPLIT], out_dt, tag="ot")
                nc.vector.tensor_copy(ot[:, :nsz], ps[:, :nsz])
                # Alternate output queues: parallel trigger streams and a
                # two-queue final drain.
                if OUT_Q == "alt":
                    q = nc.sync if evict_n[0] % 2 == 0 else nc.scalar
                else:
                    q = nc.sync
                evict_n[0] += 1
                q.dma_start(yt[mi, :, n0 : n0 + nsz], ot[:, :nsz])

            # First n-tile: PASSES k-passes across all m, tracking the
            # arriving W/X0 chunks; evict on the last pass.
            ps0 = []
            for c in range(PASSES):
                for mi in range(MT):
                    if c == 0:
                        ps = psum_pool.tile([P, NSPLIT], mybir.dt.float32, tag="ps")
                        ps0.append(ps)
                    ps = ps0[mi]
                    for k in range(c * KC, (c + 1) * KC):
                        mm(
                            ps,
                            mi,
                            0,
                            k,
                            nsz_first,
                            start=(k == 0),
                            stop=(k == KT - 1),
                        )
                    if c == PASSES - 1:
                        evict(ps, mi, n0_first, nsz_first)
            # Remaining n-tiles: fused k loop.
            for j, (n0, nsz) in enumerate(n_tiles):
                if j == 0:
                    continue
                for mi in range(MT):
                    ps = psum_pool.tile([P, NSPLIT], mybir.dt.float32, tag="ps")
                    for k in range(KT):
                        mm(ps, mi, j, k, nsz, start=(k == 0), stop=(k == KT - 1))
                    evict(ps, mi, n0, nsz)
            if WARMUP_MM and TAIL_MM:
                # Keep the PE active through the output-DMA drain so the HAM
                # clock gate stays at 8/8 for the runtime's teardown sequence.
                tp = psum_pool.tile([P, NSPLIT], mybir.dt.float32, tag="ps")
                for i in range(TAIL_MM):
                    nc.tensor.matmul(
                        tp[:], lhsT=wt[:, :P], rhs=wt[:], start=True, stop=True
                    )
    nc.compile()
    return nc


def kernel(hidden: np.ndarray, gate_logits: np.ndarray, W: np.ndarray) -> np.ndarray:
    from concourse.bass_utils import run_bass_kernel_spmd

    hidden = np.asarray(hidden)
    gate_logits = np.asarray(gate_logits)
    W = np.asarray(W)
    B, S, D = hidden.shape
    T, E = gate_logits.shape
    assert E == N_CORES
    x = np.ascontiguousarray(hidden.reshape(T, D).astype(np.float32))

    # --- routing on host (fp32, matches reference softmax/top-2) ---
    g = gate_logits.astype(np.float32)
    m = g.max(axis=-1, keepdims=True)
    p = np.exp(g - m)
    p /= p.sum(axis=-1, keepdims=True)
    top2 = np.argpartition(-p, 1, axis=-1)[:, :2]

    routed = [np.nonzero((top2 == e).any(axis=1))[0] for e in range(E)]
    counts = np.array([len(r) for r in routed])
    if C_MODE == "exact":
        C = max(NMIN, int(counts.max()))
    else:
        C = max(NMIN, int(-(-counts.max() // P)) * P)

    mm_np = _np_dt(MM_DT)
    KT = D // P
    n_tiles = _n_tiles(C)
    NT = len(n_tiles)

    in_maps = []
    for e in range(E):
        idx = routed[e]
        scale = p[idx, e].astype(np.float32)
        xe = x[idx] * scale[:, None]  # [cnt, D]
        xt_full = np.zeros((D, C), dtype=mm_np)
        xt_full[:, : len(idx)] = xe.T.astype(mm_np)
        # [D, C] -> [KT, P, C] -> per-n-tile [NT, P, KT, nsz] contiguous
        xk = xt_full.reshape(KT, P, C)
        xt_dram = np.zeros((NT, P, KT * NSPLIT), dtype=mm_np)
        for j, (n0, nsz) in enumerate(n_tiles):
            blk = xk[:, :, n0 : n0 + nsz].transpose(1, 0, 2)  # [P, KT, nsz]
            xt_dram[j, :, : KT * nsz] = blk.reshape(P, KT * nsz)
        w_full = (
            W[e].astype(mm_np).reshape(KT, P, D).transpose(1, 0, 2).reshape(P, KT * D)
        )
        in_maps.append(
            {
                "xt": np.ascontiguousarray(xt_dram),
                "w": np.ascontiguousarray(w_full),
            }
        )

    key = (D, C, MM_DT, OUT_DT, WARMUP_MM, TAIL_MM, OUT_Q, NT_MODE, WARM_INIT)
    if key not in _prog_cache:
        _prog_cache[key] = _build_program(D, C, MM_DT, OUT_DT)
    nc = _prog_cache[key]

    res = run_bass_kernel_spmd(nc, in_maps, core_ids=list(range(N_CORES)))

    # --- combine on host ---
    out = np.zeros((T, D), dtype=np.float32)
    for e in range(E):
        idx = routed[e]
        ye_t = res.results[e]["yt"].reshape(D, C)  # Y^T
        out[idx] += ye_t[:, : len(idx)].T.astype(np.float32)
    return out.reshape(B, S, D)

